# revision 17
# baseline (speedup 1.0000x reference)
"""BinaryLinear Trainium2 kernel: out = sign(x) @ sign(W).T

x: (4, 4096, 1024) f32, W: (1024, 1024) f32 -> out (4, 4096, 1024) f32.

Strategy (8 NeuronCores, data-parallel over flattened batch*seq):
  - Each core gets a [2048, 1024] row-shard of x and the full W.
  - x is re-laid-out on the host (pure permutation, no arithmetic) so the
    contraction index i lands on SBUF partitions directly: per core the DRAM
    tensor is [8 chunks * 128 p, (4 j, 2 c, 2 t', 128 u)] f32 with
    i = 256 j + 128 c + p and row m = 512 g + 4 u + 2 b0 + t' for chunk
    ch = 2 g + b0. This removes the on-chip transpose entirely and loads
    with 8 KiB-per-partition contiguous descriptors.
  - DMA descriptor generation is serialized per queue (~110-140 GB/s per
    queue), so each 1 MiB chunk load is split into 4 sub-DMAs pinned to the
    4 SWDGE queues (qPoolDynamic..qPoolDynamic3). Each queue is FIFO, so
    chunk completions stay ordered without explicit chaining while all 4
    descgen streams run in parallel (~450 GB/s issue capability, HBM-bound).
  - Per chunk (256 rows): ACT Sign (f32 -> fp8e4, +-1/0 exact) -> 16 fp8
    DoubleRow matmuls (K=256 each) accumulating [128 m, 512 o] PSUM tiles
    -> DVE copy PSUM -> SBUF as float16.
  - Outputs are exact integers |v| <= 1024, representable exactly in fp16,
    so stores are half-width; the host upcasts to f32. The 4-way row
    interleave (m = 512g + 4u + b) makes each partition hold 4 adjacent
    DRAM rows, i.e. 8 KiB store descriptors. Stores cover one 512-row
    group (two chunks) and are split into 2 sub-DMAs on the SP and
    Activation HWDGE queues so they never block a load trigger, with the
    Act-queue store triggers reordered after all signs so a store's
    copy-wait can never stall a sign.
  - W is repacked once on the host: wq[p, (j, c, o)] = sign(W)[o, i] fp8;
    loaded as 2 half-DMAs on the SP/Act HWDGE queues concurrently with the
    first x chunk. A dummy 1-element Sign activation with no dependencies
    preloads the ACT function table during the preamble.

All arithmetic is exact: sign values are +-1/0 (exact in fp8e4), the PE
accumulates in fp32, and |out| <= 1024 is exact in fp16.
"""

import numpy as np

P = 128
K = 1024  # in_features
N = 1024  # out_features
N_CORES = 8
M_TOTAL = 4 * 4096
M_PER_CORE = M_TOTAL // N_CORES
MC = 128  # rows per chunk
N_CH = M_PER_CORE // MC
N_GRP = N_CH // 4  # 512-row store groups (4 chunks each)
X_BUFS = 16


def build_binary_linear(tc, out, x, w):
    """Emit the per-core Tile kernel.

    out: DRAM [M_PER_CORE, N] f16, x: DRAM [N_CH*P, 8*MC] f32 (host-packed),
    w: DRAM [P, 8*N] fp8 (host-packed).
    """
    import concourse.mybir as mybir

    nc = tc.nc
    f32 = mybir.dt.float32
    f16 = mybir.dt.float16
    fp8 = mybir.dt.float8e4
    Sign = mybir.ActivationFunctionType.Sign
    DR = mybir.MatmulPerfMode.DoubleRow

    with (
        tc.tile_pool(name="wsb", bufs=1) as wpool,
        tc.tile_pool(name="xin", bufs=X_BUFS) as xin_pool,
        tc.tile_pool(name="x8p", bufs=4) as x8_pool,
        tc.tile_pool(name="osb", bufs=4) as out_pool,
        tc.tile_pool(name="ps", bufs=4, space="PSUM") as psum_pool,
    ):
        # Preload the ACT Sign table during the preamble: a 1-partition,
        # 8-element Sign with no data dependencies.
        dumf = wpool.tile([1, 8], f32, name="dumf")
        dum8 = wpool.tile([1, 8], fp8, name="dum8")
        nc.vector.memset(dumf, 0.0)
        nc.scalar.activation(out=dum8, in_=dumf, func=Sign)

        # ---- W: host-packed fp8 [128, 8*1024]; wq[p, (j, c, o)]
        # = sign(W)[o, i] with i = 256j + 128c + p. Two half-DMAs on the
        # SP / Act HWDGE queues. ----
        wT = wpool.tile([P, 8 * N], fp8, name="wT")
        nc.sync.dma_start(out=wT[: P // 2, :], in_=w[: P // 2, :])
        nc.scalar.dma_start(out=wT[P // 2 :, :], in_=w[P // 2 :, :])
        w4 = wT.rearrange("p (j c o) -> p j c o", j=4, c=2)

        osbs = {}
        for ch in range(N_CH):
            g, b0 = divmod(ch, 4)
            xf = xin_pool.tile([P, 8 * MC], f32, tag="xf", name=f"xf{ch}")
            inst = nc.gpsimd.dma_start(out=xf, in_=x[ch * P : (ch + 1) * P, :])
            qn = 3 if ch % 8 == 7 else ch % 3
            inst.ins.queue = f"qPoolDynamic{qn or ''}"
            x8 = x8_pool.tile([P, 8 * MC], fp8, tag="x8", name=f"x8{ch}")
            nc.scalar.activation(out=x8, in_=xf, func=Sign)
            x84 = x8.rearrange("p (j c m) -> p j c m", j=4, c=2)

            if b0 == 0:
                osbs[g] = out_pool.tile([P, 4 * N], f16, tag="osb", name=f"osb{g}")
            osb2 = osbs[g].rearrange("p (b o) -> p b o", b=4)
            ps = [
                psum_pool.tile([P, 512], f32, tag=f"ps{h}", name=f"ps{h}")
                for h in range(2)
            ]
            for j in range(4):
                lhsT = x84[:, j, :, :]
                for h in range(2):
                    nc.tensor.matmul(
                        ps[h],
                        lhsT=lhsT,
                        rhs=w4[:, j, :, h * 512 : (h + 1) * 512],
                        start=(j == 0),
                        stop=(j == 3),
                        perf_mode=DR,
                    )
            for h in range(2):
                nc.vector.tensor_copy(
                    out=osb2[:, b0, h * 512 : (h + 1) * 512], in_=ps[h]
                )
            # Stores: mid-pipeline groups go on the Pool SWDGE queues as one
            # full-group DMA pair (trigger ~0.6us, descgen offloaded; the
            # ~9.5us completion latency hides behind the PE-paced pipeline).
            # The LAST group's store latency is exposed, so it goes on the
            # engine-synchronous SP/Act HWDGE queues in two b-halves: rows
            # of chunks 12-13 right after their copies (~5us before the
            # end), rows of chunks 14-15 right after the final copies.
            if b0 == 3 and g < N_GRP - 1:
                for q in range(2):
                    r0 = 512 * g + 256 * q
                    o_ap = out[r0 : r0 + 256].rearrange("(p b) o -> p (b o)", b=4)
                    i_ap = osbs[g][64 * q : 64 * (q + 1), :]
                    inst = nc.gpsimd.dma_start(out=o_ap, in_=i_ap)
                    qn = {(0, 0): 3, (0, 1): 3, (1, 0): 0, (1, 1): 1,
                          (2, 0): 2, (2, 1): 3}[(g, q)]
                    inst.ins.queue = f"qPoolDynamic{qn or ''}"
            elif b0 in (1, 3) and g == N_GRP - 1:
                bsl = slice(0, 2) if b0 == 1 else slice(2, 4)
                for q in range(2):
                    r0 = 512 * g + 256 * q
                    o_ap = out[r0 : r0 + 256].rearrange(
                        "(p bb) o -> p bb o", bb=4
                    )[:, bsl, :]
                    i_ap = osbs[g][
                        64 * q : 64 * (q + 1), 2048 * (b0 // 2) : 2048 * (b0 // 2 + 1)
                    ]
                    (nc.sync, nc.scalar)[q].dma_start(out=o_ap, in_=i_ap)


def _rewire_waits(nc):
    """Reorder Act-queue store triggers after all signs, then replace Tile's
    conservative / lane-aliased DMA waits with exact producer-based waits.

      xf[ch]     <- sign[ch - X_BUFS] (xf-slot WAR)
      w halves   <- (nothing; first on their HWDGE queues)
      sign[ch]   <- all 4 xf[ch] sub completions (RAW) + keep Tile's PE
                    wait (x8-slot WAR)
      copy[...]  <- keep Tile's PE wait only (psum RAW; osb pool has one
                    buffer per group, no WAR)
      store[g,q] <- last copy of group g (RAW)

    Waits are emitted as (producer's update-sem >= cumulative value after
    it); lane-order waits keep same-sem DMA updates ordered so >= waits
    cannot be satisfied by a later DMA that shares the semaphore.
    """
    import concourse.mybir as mybir

    # -- pass 0a: move Act-engine store DMAs after the last InstActivation --
    for f in nc.m.functions:
        for bb in f.blocks:
            ins_list = bb.instructions
            act_stores = [
                i
                for i in ins_list
                if type(i).__name__ == "InstDMACopy"
                and str(i.engine).endswith("Activation")
                and str(i.outs[0].memref).startswith("out")
            ]
            if not act_stores:
                continue
            rest = [i for i in ins_list if i not in act_stores]
            last_act = max(
                idx
                for idx, i in enumerate(rest)
                if type(i).__name__ == "InstActivation"
            )
            bb.instructions[:] = (
                rest[: last_act + 1] + act_stores + rest[last_act + 1 :]
            )

    # -- pass 0b: reposition Pool-queue store DMAs right after the load
    # trigger whose slot-WAR wait is looser than the store's copy-wait
    # (store[g] after xf[4g+10]), so they never head-of-line block a load --
    for f in nc.m.functions:
        for bb in f.blocks:
            ins_list = bb.instructions
            pool_stores = {}
            for i in ins_list:
                if (
                    type(i).__name__ == "InstDMACopy"
                    and str(i.engine).endswith("Pool")
                    and str(i.outs[0].memref).startswith("out")
                ):
                    g = int(i.outs[0].offset) // (512 * N)
                    pool_stores.setdefault(g, []).append(i)
            if not pool_stores:
                continue
            flat = [i for v in pool_stores.values() for i in v]
            rest = [i for i in ins_list if i not in flat]
            xf_pos = {}
            for idx, i in enumerate(rest):
                if type(i).__name__ == "InstDMACopy" and str(
                    i.outs[0].memref
                ).startswith("xf"):
                    ch = int(str(i.outs[0].memref)[2:].split("_")[0])
                    xf_pos[ch] = idx
            inserts = {}  # position -> [insts]
            last_xf = max(xf_pos.values())
            for g in sorted(pool_stores):
                inserts.setdefault(last_xf, []).extend(pool_stores[g])
            new_list = []
            for idx, i in enumerate(rest):
                new_list.append(i)
                if idx in inserts:
                    new_list.extend(inserts[idx])
            bb.instructions[:] = new_list

    insts = []
    for f in nc.m.functions:
        for bb in f.blocks:
            insts.extend(bb.instructions)

    cum = {}
    upd_after = {}  # inst name -> (sem_name, sem_id, cum_value_after)
    lane_order = {}  # inst name -> SyncWait enforcing same-lane completion order
    xf_subs = {}  # ch -> [inst]
    signs = {}  # ch -> inst
    copies = {}  # g -> [inst]
    stores = {}  # g -> [inst]
    w_loads = []
    for ins in insts:
        si = getattr(ins, "sync_info", None)
        if si is None:
            continue
        for u in si.on_update or []:
            prev = cum.get(u.ant_name, 0)
            if prev > 0 and (
                u.ant_name.startswith("DMAHW") or u.ant_name.startswith("DMASW")
            ):
                lane_order[ins.name] = mybir.SyncWait(
                    sync_type="semaphore",
                    id=u.id,
                    ant_name=u.ant_name,
                    wait_mode="sem-ge-imm",
                    wait_value=prev,
                )
            cum[u.ant_name] = prev + u.update_value
            upd_after[ins.name] = (u.ant_name, u.id, cum[u.ant_name])
        memref = str(getattr(ins.outs[0], "memref", "")) if ins.outs else ""
        tn = type(ins).__name__
        if tn == "InstDMACopy" and memref.startswith("xf"):
            ch = int(memref[2 : memref.index("_")])
            xf_subs.setdefault(ch, []).append(ins)
        elif tn == "InstDMACopy" and memref.startswith("wT"):
            w_loads.append(ins)
        elif tn == "InstDMACopy" and memref.startswith("out"):
            off = int(ins.outs[0].offset)  # in f16 elements
            g = off // (512 * N)
            stores.setdefault(g, []).append(ins)
        elif tn == "InstActivation" and memref.startswith("x8"):
            ch = int(memref[2 : memref.index("_")])
            signs[ch] = ins
        elif tn == "InstTensorCopy" and memref.startswith("osb"):
            g = int(memref[3 : memref.index("_")])
            copies.setdefault(g, []).append(ins)

    assert sorted(xf_subs) == list(range(N_CH)) and all(
        len(v) == 1 for v in xf_subs.values()
    ), {k: len(v) for k, v in xf_subs.items()}
    assert sorted(signs) == list(range(N_CH))
    assert sorted(copies) == list(range(N_GRP)) and all(
        len(v) == 8 for v in copies.values()
    )
    assert sorted(stores) == list(range(N_GRP)) and all(
        len(stores[g]) == (4 if g == N_GRP - 1 else 2) for g in stores
    ), {k: len(v) for k, v in stores.items()}
    assert len(w_loads) == 2

    def wait_on(producer_ins):
        sem_name, sem_id, v = upd_after[producer_ins.name]
        return mybir.SyncWait(
            sync_type="semaphore",
            id=sem_id,
            ant_name=sem_name,
            wait_mode="sem-ge-imm",
            wait_value=v,
        )

    def keep_engine_waits(ins):
        return [
            w
            for w in (ins.sync_info.on_wait or [])
            if not (
                w.ant_name.startswith("DMAHW")
                or w.ant_name.startswith("DMASW")
                or w.ant_name.startswith("Activation")
                or w.ant_name.startswith("DVE")
            )
        ]

    def set_waits(ins, producers, extra=()):
        si = ins.sync_info
        waits = [wait_on(p) for p in producers if p is not None] + list(extra)
        lo = lane_order.get(ins.name)
        if lo is not None:
            waits.append(lo)
        ins.sync_info = mybir.SyncInfo(
            on_wait=waits, on_update=list(si.on_update or [])
        )

    for ch in range(N_CH):
        for ins in xf_subs[ch]:
            set_waits(ins, [signs[ch - X_BUFS]] if ch >= X_BUFS else [])
    for ins in w_loads:
        set_waits(ins, [])
    for ch in range(N_CH):
        set_waits(signs[ch], xf_subs[ch], extra=keep_engine_waits(signs[ch]))
    for g in range(N_GRP):
        for ins in copies[g]:
            set_waits(ins, [], extra=keep_engine_waits(ins))
        for ins in stores[g]:
            if g == N_GRP - 1:
                half = ((int(ins.outs[0].offset) // N) % 4) // 2
                dep = copies[g][3] if half == 0 else copies[g][7]
            else:
                dep = copies[g][-1]
            set_waits(ins, [dep])


def _legalize_dma_waits(nc):
    """Walrus caps in-struct sem waits (DMA_DIRECT2D takes 1, DMACopy 2).

    Tile's sem assignment is not transitively minimal and can emit 2-4 waits
    on DMA instructions. Hoist the excess into InstEventSemaphore wait-only
    instructions inserted just before the DMA on its triggering queue. This
    is sound: the queue executes the hoisted wait strictly before pushing the
    DMA descriptor, so the dependency is enforced (more conservatively) at
    trigger time instead of ring-pop time.
    """
    import concourse.mybir as mybir

    limits = {
        "InstDmaTransposeAnt": 1,
        "InstDMACopy": 1,
        "InstTensorCopy": 1,
        "InstActivation": 1,
        "InstMatmult": 1,
        "InstLdweights": 1,
        "InstMemset": 1,
        "InstTensorTensor": 1,
        "InstDrain": 1,
    }
    n_hoisted = 0
    for f in nc.m.functions:
        for bb in f.blocks:
            new_list = []
            for ins in bb.instructions:
                lim = limits.get(type(ins).__name__)
                si = getattr(ins, "sync_info", None)
                waits = list(si.on_wait) if si is not None and si.on_wait else []
                if lim is not None and len(waits) > lim:
                    # keep data-producer (engine-sem) waits in-struct first,
                    # then the freshest DMA-lane waits; hoist the rest
                    def keep_rank(w):
                        is_lane = w.ant_name.startswith(
                            "DMAHW"
                        ) or w.ant_name.startswith("DMASW")
                        return (1 if is_lane else 0, -w.wait_value)

                    waits_sorted = sorted(waits, key=keep_rank)
                    keep, hoist = waits_sorted[:lim], waits_sorted[lim:]
                    for ci in range(0, len(hoist), 2):
                        chunk = hoist[ci : ci + 2]
                        ev = mybir.InstEventSemaphore(
                            name=f"{ins.name}-prewait{ci // 2}",
                            engine=ins.engine,
                            ins=[],
                            outs=[],
                            sync_info=mybir.SyncInfo(on_wait=chunk, on_update=[]),
                        )
                        nc.inst_map[ev.name] = ev
                        new_list.append(ev)
                        n_hoisted += len(chunk)
                    ins.sync_info = mybir.SyncInfo(
                        on_wait=keep, on_update=list(si.on_update or [])
                    )
                new_list.append(ins)
            bb.instructions[:] = new_list
    return n_hoisted


def _build_nc():
    import concourse.bass as bass
    import concourse.mybir as mybir
    from concourse import tile

    nc = bass.Bass("TRN2", target_bir_lowering=False, num_swdge_queues=4)
    x_d = nc.dram_tensor(
        "x", [N_CH * P, 8 * MC], mybir.dt.float32, kind="ExternalInput"
    )
    w_d = nc.dram_tensor("W", [P, 8 * N], mybir.dt.float8e4, kind="ExternalInput")
    out_d = nc.dram_tensor(
        "out", [M_PER_CORE, N], mybir.dt.float16, kind="ExternalOutput"
    )
    with tile.TileContext(nc) as tc:
        build_binary_linear(tc, out_d.ap(), x_d.ap(), w_d.ap())
    _rewire_waits(nc)
    _legalize_dma_waits(nc)
    return nc


_cached = {}


def _get_nc():
    if "nc" not in _cached:
        _cached["nc"] = _build_nc()
    return _cached["nc"]


def kernel(x, W, _trace=False):
    from concourse import bass_utils

    import ml_dtypes

    xf = np.asarray(x, dtype=np.float32).reshape(M_TOTAL, K)
    # host re-layout (pure permutation): per core [ (g, b0, p), (j, c, u) ]
    # with m = 2048*core + 512g + 4u + b0 and i = 256j + 128c + p
    T = xf.reshape(N_CORES, 4, P, 4, 4, 2, P)  # (core, g, u, b0, j, c, p)
    xh = np.ascontiguousarray(T.transpose(0, 1, 3, 6, 4, 5, 2)).reshape(
        N_CORES, N_CH * P, 8 * MC
    )
    # pack sign(W) fp8: wq[p, (j, c, o)] = sign(W)[o, 256j + 128c + p]
    sT = np.sign(np.asarray(W, dtype=np.float32)).T.astype(ml_dtypes.float8_e4m3)
    wq = np.ascontiguousarray(
        sT.reshape(4, 2, P, N).transpose(2, 0, 1, 3)
    ).reshape(P, 8 * N)
    in_maps = [{"x": xh[i], "W": wq} for i in range(N_CORES)]
    nc = _get_nc()
    res = bass_utils.run_bass_kernel_spmd(
        nc, in_maps, core_ids=list(range(N_CORES)), trace=_trace
    )
    out = np.concatenate([r["out"] for r in res.results], axis=0)
    out = out.astype(np.float32).reshape(4, 4096, N)
    if _trace:
        kernel.last_results = res
    return out


# revision 18
# speedup vs baseline: 1.0122x; 1.0122x over previous
"""BinaryLinear Trainium2 kernel: out = sign(x) @ sign(W).T

x: (4, 4096, 1024) f32, W: (1024, 1024) f32 -> out (4, 4096, 1024) f32.

Strategy (8 NeuronCores, data-parallel over flattened batch*seq):
  - Each core gets a [2048, 1024] row-shard of x and the full W.
  - x is re-laid-out on the host (pure permutation, no arithmetic) so the
    contraction index i lands on SBUF partitions directly: per core the DRAM
    tensor is [8 chunks * 128 p, (4 j, 2 c, 2 t', 128 u)] f32 with
    i = 256 j + 128 c + p and row m = 512 g + 4 u + 2 b0 + t' for chunk
    ch = 2 g + b0. This removes the on-chip transpose entirely and loads
    with 8 KiB-per-partition contiguous descriptors.
  - DMA descriptor generation is serialized per queue (~110-140 GB/s per
    queue), so each 1 MiB chunk load is split into 4 sub-DMAs pinned to the
    4 SWDGE queues (qPoolDynamic..qPoolDynamic3). Each queue is FIFO, so
    chunk completions stay ordered without explicit chaining while all 4
    descgen streams run in parallel (~450 GB/s issue capability, HBM-bound).
  - Per chunk (256 rows): ACT Sign (f32 -> fp8e4, +-1/0 exact) -> 16 fp8
    DoubleRow matmuls (K=256 each) accumulating [128 m, 512 o] PSUM tiles
    -> DVE copy PSUM -> SBUF as float16.
  - Outputs are exact integers |v| <= 1024, representable exactly in fp16,
    so stores are half-width; the host upcasts to f32. The 4-way row
    interleave (m = 512g + 4u + b) makes each partition hold 4 adjacent
    DRAM rows, i.e. 8 KiB store descriptors. Stores cover one 512-row
    group (two chunks) and are split into 2 sub-DMAs on the SP and
    Activation HWDGE queues so they never block a load trigger, with the
    Act-queue store triggers reordered after all signs so a store's
    copy-wait can never stall a sign.
  - W is repacked once on the host: wq[p, (j, c, o)] = sign(W)[o, i] fp8;
    loaded as 2 half-DMAs on the SP/Act HWDGE queues concurrently with the
    first x chunk. A dummy 1-element Sign activation with no dependencies
    preloads the ACT function table during the preamble.

All arithmetic is exact: sign values are +-1/0 (exact in fp8e4), the PE
accumulates in fp32, and |out| <= 1024 is exact in fp16.
"""

import numpy as np

P = 128
K = 1024  # in_features
N = 1024  # out_features
N_CORES = 8
M_TOTAL = 4 * 4096
M_PER_CORE = M_TOTAL // N_CORES
MC = 128  # rows per chunk
N_CH = M_PER_CORE // MC
N_GRP = N_CH // 4  # 512-row store groups (4 chunks each)
X_BUFS = 16


def build_binary_linear(tc, out, x, w):
    """Emit the per-core Tile kernel.

    out: DRAM [M_PER_CORE, N] f16, x: DRAM [N_CH*P, 8*MC] f32 (host-packed),
    w: DRAM [P, 8*N] fp8 (host-packed).
    """
    import concourse.mybir as mybir

    nc = tc.nc
    f32 = mybir.dt.float32
    f16 = mybir.dt.float16
    fp8 = mybir.dt.float8e4
    Sign = mybir.ActivationFunctionType.Sign
    DR = mybir.MatmulPerfMode.DoubleRow

    with (
        tc.tile_pool(name="wsb", bufs=1) as wpool,
        tc.tile_pool(name="xin", bufs=X_BUFS) as xin_pool,
        tc.tile_pool(name="x8p", bufs=4) as x8_pool,
        tc.tile_pool(name="osb", bufs=4) as out_pool,
        tc.tile_pool(name="ps", bufs=4, space="PSUM") as psum_pool,
    ):
        # Preload the ACT Sign table during the preamble: a 1-partition,
        # 8-element Sign with no data dependencies.
        dumf = wpool.tile([1, 8], f32, name="dumf")
        dum8 = wpool.tile([1, 8], fp8, name="dum8")
        nc.vector.memset(dumf, 0.0)
        nc.scalar.activation(out=dum8, in_=dumf, func=Sign)

        # ---- W: host-packed fp8 [128, 8*1024]; wq[p, (j, c, o)]
        # = sign(W)[o, i] with i = 256j + 128c + p. Two half-DMAs on the
        # SP / Act HWDGE queues. ----
        wT = wpool.tile([P, 8 * N], fp8, name="wT")
        nc.sync.dma_start(out=wT[: P // 2, :], in_=w[: P // 2, :])
        nc.scalar.dma_start(out=wT[P // 2 :, :], in_=w[P // 2 :, :])
        w4 = wT.rearrange("p (j c o) -> p j c o", j=4, c=2)

        osbs = {}
        for ch in range(N_CH):
            g, b0 = divmod(ch, 4)
            xf = xin_pool.tile([P, 8 * MC], f32, tag="xf", name=f"xf{ch}")
            inst = nc.gpsimd.dma_start(out=xf, in_=x[ch * P : (ch + 1) * P, :])
            qn = 3 if ch % 8 == 7 else ch % 3
            inst.ins.queue = f"qPoolDynamic{qn or ''}"
            x8 = x8_pool.tile([P, 8 * MC], fp8, tag="x8", name=f"x8{ch}")
            nc.scalar.activation(out=x8, in_=xf, func=Sign)
            x84 = x8.rearrange("p (j c m) -> p j c m", j=4, c=2)

            if b0 == 0:
                osbs[g] = out_pool.tile([P, 4 * N], f16, tag="osb", name=f"osb{g}")
            osb2 = osbs[g].rearrange("p (b o) -> p b o", b=4)
            ps = [
                psum_pool.tile([P, 512], f32, tag=f"ps{h}", name=f"ps{h}")
                for h in range(2)
            ]
            for j in range(4):
                lhsT = x84[:, j, :, :]
                for h in range(2):
                    nc.tensor.matmul(
                        ps[h],
                        lhsT=lhsT,
                        rhs=w4[:, j, :, h * 512 : (h + 1) * 512],
                        start=(j == 0),
                        stop=(j == 3),
                        perf_mode=DR,
                    )
            for h in range(2):
                nc.vector.tensor_copy(
                    out=osb2[:, b0, h * 512 : (h + 1) * 512], in_=ps[h]
                )
            # Stores: mid-pipeline groups go on the Pool SWDGE queues as one
            # full-group DMA pair (trigger ~0.6us, descgen offloaded; the
            # ~9.5us completion latency hides behind the PE-paced pipeline).
            # The LAST group's store latency is exposed, so it goes on the
            # engine-synchronous SP/Act HWDGE queues in two b-halves: rows
            # of chunks 12-13 right after their copies (~5us before the
            # end), rows of chunks 14-15 right after the final copies.
            if b0 == 3 and g < N_GRP - 2:
                for q in range(2):
                    r0 = 512 * g + 256 * q
                    o_ap = out[r0 : r0 + 256].rearrange("(p b) o -> p (b o)", b=4)
                    i_ap = osbs[g][64 * q : 64 * (q + 1), :]
                    inst = nc.gpsimd.dma_start(out=o_ap, in_=i_ap)
                    qn = {(0, 0): 3, (0, 1): 3, (1, 0): 0, (1, 1): 1}[(g, q)]
                    inst.ins.queue = f"qPoolDynamic{qn or ''}"
            elif b0 in (1, 3) and g >= N_GRP - 2:
                bsl = slice(0, 2) if b0 == 1 else slice(2, 4)
                for q in range(2):
                    r0 = 512 * g + 256 * q
                    o_ap = out[r0 : r0 + 256].rearrange(
                        "(p bb) o -> p bb o", bb=4
                    )[:, bsl, :]
                    i_ap = osbs[g][
                        64 * q : 64 * (q + 1), 2048 * (b0 // 2) : 2048 * (b0 // 2 + 1)
                    ]
                    (nc.sync, nc.scalar)[q].dma_start(out=o_ap, in_=i_ap)


def _rewire_waits(nc):
    """Reorder Act-queue store triggers after all signs, then replace Tile's
    conservative / lane-aliased DMA waits with exact producer-based waits.

      xf[ch]     <- sign[ch - X_BUFS] (xf-slot WAR)
      w halves   <- (nothing; first on their HWDGE queues)
      sign[ch]   <- all 4 xf[ch] sub completions (RAW) + keep Tile's PE
                    wait (x8-slot WAR)
      copy[...]  <- keep Tile's PE wait only (psum RAW; osb pool has one
                    buffer per group, no WAR)
      store[g,q] <- last copy of group g (RAW)

    Waits are emitted as (producer's update-sem >= cumulative value after
    it); lane-order waits keep same-sem DMA updates ordered so >= waits
    cannot be satisfied by a later DMA that shares the semaphore.
    """
    import concourse.mybir as mybir

    # -- pass 0a: move Act-engine store DMAs after the last InstActivation --
    for f in nc.m.functions:
        for bb in f.blocks:
            ins_list = bb.instructions
            act_stores = [
                i
                for i in ins_list
                if type(i).__name__ == "InstDMACopy"
                and str(i.engine).endswith("Activation")
                and str(i.outs[0].memref).startswith("out")
            ]
            if not act_stores:
                continue
            rest = [i for i in ins_list if i not in act_stores]
            last_act = max(
                idx
                for idx, i in enumerate(rest)
                if type(i).__name__ == "InstActivation"
            )
            bb.instructions[:] = (
                rest[: last_act + 1] + act_stores + rest[last_act + 1 :]
            )

    # -- pass 0b: reposition Pool-queue store DMAs right after the load
    # trigger whose slot-WAR wait is looser than the store's copy-wait
    # (store[g] after xf[4g+10]), so they never head-of-line block a load --
    for f in nc.m.functions:
        for bb in f.blocks:
            ins_list = bb.instructions
            pool_stores = {}
            for i in ins_list:
                if (
                    type(i).__name__ == "InstDMACopy"
                    and str(i.engine).endswith("Pool")
                    and str(i.outs[0].memref).startswith("out")
                ):
                    g = int(i.outs[0].offset) // (512 * N)
                    pool_stores.setdefault(g, []).append(i)
            if not pool_stores:
                continue
            flat = [i for v in pool_stores.values() for i in v]
            rest = [i for i in ins_list if i not in flat]
            xf_pos = {}
            for idx, i in enumerate(rest):
                if type(i).__name__ == "InstDMACopy" and str(
                    i.outs[0].memref
                ).startswith("xf"):
                    ch = int(str(i.outs[0].memref)[2:].split("_")[0])
                    xf_pos[ch] = idx
            inserts = {}  # position -> [insts]
            last_xf = max(xf_pos.values())
            for g in sorted(pool_stores):
                inserts.setdefault(last_xf, []).extend(pool_stores[g])
            new_list = []
            for idx, i in enumerate(rest):
                new_list.append(i)
                if idx in inserts:
                    new_list.extend(inserts[idx])
            bb.instructions[:] = new_list

    insts = []
    for f in nc.m.functions:
        for bb in f.blocks:
            insts.extend(bb.instructions)

    cum = {}
    upd_after = {}  # inst name -> (sem_name, sem_id, cum_value_after)
    lane_order = {}  # inst name -> SyncWait enforcing same-lane completion order
    xf_subs = {}  # ch -> [inst]
    signs = {}  # ch -> inst
    copies = {}  # g -> [inst]
    stores = {}  # g -> [inst]
    w_loads = []
    for ins in insts:
        si = getattr(ins, "sync_info", None)
        if si is None:
            continue
        for u in si.on_update or []:
            prev = cum.get(u.ant_name, 0)
            if prev > 0 and (
                u.ant_name.startswith("DMAHW") or u.ant_name.startswith("DMASW")
            ):
                lane_order[ins.name] = mybir.SyncWait(
                    sync_type="semaphore",
                    id=u.id,
                    ant_name=u.ant_name,
                    wait_mode="sem-ge-imm",
                    wait_value=prev,
                )
            cum[u.ant_name] = prev + u.update_value
            upd_after[ins.name] = (u.ant_name, u.id, cum[u.ant_name])
        memref = str(getattr(ins.outs[0], "memref", "")) if ins.outs else ""
        tn = type(ins).__name__
        if tn == "InstDMACopy" and memref.startswith("xf"):
            ch = int(memref[2 : memref.index("_")])
            xf_subs.setdefault(ch, []).append(ins)
        elif tn == "InstDMACopy" and memref.startswith("wT"):
            w_loads.append(ins)
        elif tn == "InstDMACopy" and memref.startswith("out"):
            off = int(ins.outs[0].offset)  # in f16 elements
            g = off // (512 * N)
            stores.setdefault(g, []).append(ins)
        elif tn == "InstActivation" and memref.startswith("x8"):
            ch = int(memref[2 : memref.index("_")])
            signs[ch] = ins
        elif tn == "InstTensorCopy" and memref.startswith("osb"):
            g = int(memref[3 : memref.index("_")])
            copies.setdefault(g, []).append(ins)

    assert sorted(xf_subs) == list(range(N_CH)) and all(
        len(v) == 1 for v in xf_subs.values()
    ), {k: len(v) for k, v in xf_subs.items()}
    assert sorted(signs) == list(range(N_CH))
    assert sorted(copies) == list(range(N_GRP)) and all(
        len(v) == 8 for v in copies.values()
    )
    assert sorted(stores) == list(range(N_GRP)) and all(
        len(stores[g]) == (4 if g >= N_GRP - 2 else 2) for g in stores
    ), {k: len(v) for k, v in stores.items()}
    assert len(w_loads) == 2

    def wait_on(producer_ins):
        sem_name, sem_id, v = upd_after[producer_ins.name]
        return mybir.SyncWait(
            sync_type="semaphore",
            id=sem_id,
            ant_name=sem_name,
            wait_mode="sem-ge-imm",
            wait_value=v,
        )

    def keep_engine_waits(ins):
        return [
            w
            for w in (ins.sync_info.on_wait or [])
            if not (
                w.ant_name.startswith("DMAHW")
                or w.ant_name.startswith("DMASW")
                or w.ant_name.startswith("Activation")
                or w.ant_name.startswith("DVE")
            )
        ]

    def set_waits(ins, producers, extra=()):
        si = ins.sync_info
        waits = [wait_on(p) for p in producers if p is not None] + list(extra)
        lo = lane_order.get(ins.name)
        if lo is not None:
            waits.append(lo)
        ins.sync_info = mybir.SyncInfo(
            on_wait=waits, on_update=list(si.on_update or [])
        )

    for ch in range(N_CH):
        for ins in xf_subs[ch]:
            set_waits(ins, [signs[ch - X_BUFS]] if ch >= X_BUFS else [])
    for ins in w_loads:
        set_waits(ins, [])
    for ch in range(N_CH):
        set_waits(signs[ch], xf_subs[ch], extra=keep_engine_waits(signs[ch]))
    for g in range(N_GRP):
        for ins in copies[g]:
            set_waits(ins, [], extra=keep_engine_waits(ins))
        for ins in stores[g]:
            if g >= N_GRP - 2:
                half = ((int(ins.outs[0].offset) // N) % 4) // 2
                dep = copies[g][3] if half == 0 else copies[g][7]
            else:
                dep = copies[g][-1]
            set_waits(ins, [dep])


def _legalize_dma_waits(nc):
    """Walrus caps in-struct sem waits (DMA_DIRECT2D takes 1, DMACopy 2).

    Tile's sem assignment is not transitively minimal and can emit 2-4 waits
    on DMA instructions. Hoist the excess into InstEventSemaphore wait-only
    instructions inserted just before the DMA on its triggering queue. This
    is sound: the queue executes the hoisted wait strictly before pushing the
    DMA descriptor, so the dependency is enforced (more conservatively) at
    trigger time instead of ring-pop time.
    """
    import concourse.mybir as mybir

    limits = {
        "InstDmaTransposeAnt": 1,
        "InstDMACopy": 1,
        "InstTensorCopy": 1,
        "InstActivation": 1,
        "InstMatmult": 1,
        "InstLdweights": 1,
        "InstMemset": 1,
        "InstTensorTensor": 1,
        "InstDrain": 1,
    }
    n_hoisted = 0
    for f in nc.m.functions:
        for bb in f.blocks:
            new_list = []
            for ins in bb.instructions:
                lim = limits.get(type(ins).__name__)
                si = getattr(ins, "sync_info", None)
                waits = list(si.on_wait) if si is not None and si.on_wait else []
                if lim is not None and len(waits) > lim:
                    # keep data-producer (engine-sem) waits in-struct first,
                    # then the freshest DMA-lane waits; hoist the rest
                    def keep_rank(w):
                        is_lane = w.ant_name.startswith(
                            "DMAHW"
                        ) or w.ant_name.startswith("DMASW")
                        return (1 if is_lane else 0, -w.wait_value)

                    waits_sorted = sorted(waits, key=keep_rank)
                    keep, hoist = waits_sorted[:lim], waits_sorted[lim:]
                    for ci in range(0, len(hoist), 2):
                        chunk = hoist[ci : ci + 2]
                        ev = mybir.InstEventSemaphore(
                            name=f"{ins.name}-prewait{ci // 2}",
                            engine=ins.engine,
                            ins=[],
                            outs=[],
                            sync_info=mybir.SyncInfo(on_wait=chunk, on_update=[]),
                        )
                        nc.inst_map[ev.name] = ev
                        new_list.append(ev)
                        n_hoisted += len(chunk)
                    ins.sync_info = mybir.SyncInfo(
                        on_wait=keep, on_update=list(si.on_update or [])
                    )
                new_list.append(ins)
            bb.instructions[:] = new_list
    return n_hoisted


def _build_nc():
    import concourse.bass as bass
    import concourse.mybir as mybir
    from concourse import tile

    nc = bass.Bass("TRN2", target_bir_lowering=False, num_swdge_queues=4)
    x_d = nc.dram_tensor(
        "x", [N_CH * P, 8 * MC], mybir.dt.float32, kind="ExternalInput"
    )
    w_d = nc.dram_tensor("W", [P, 8 * N], mybir.dt.float8e4, kind="ExternalInput")
    out_d = nc.dram_tensor(
        "out", [M_PER_CORE, N], mybir.dt.float16, kind="ExternalOutput"
    )
    with tile.TileContext(nc) as tc:
        build_binary_linear(tc, out_d.ap(), x_d.ap(), w_d.ap())
    _rewire_waits(nc)
    _legalize_dma_waits(nc)
    return nc


_cached = {}


def _get_nc():
    if "nc" not in _cached:
        _cached["nc"] = _build_nc()
    return _cached["nc"]


def kernel(x, W, _trace=False):
    from concourse import bass_utils

    import ml_dtypes

    xf = np.asarray(x, dtype=np.float32).reshape(M_TOTAL, K)
    # host re-layout (pure permutation): per core [ (g, b0, p), (j, c, u) ]
    # with m = 2048*core + 512g + 4u + b0 and i = 256j + 128c + p
    T = xf.reshape(N_CORES, 4, P, 4, 4, 2, P)  # (core, g, u, b0, j, c, p)
    xh = np.ascontiguousarray(T.transpose(0, 1, 3, 6, 4, 5, 2)).reshape(
        N_CORES, N_CH * P, 8 * MC
    )
    # pack sign(W) fp8: wq[p, (j, c, o)] = sign(W)[o, 256j + 128c + p]
    sT = np.sign(np.asarray(W, dtype=np.float32)).T.astype(ml_dtypes.float8_e4m3)
    wq = np.ascontiguousarray(
        sT.reshape(4, 2, P, N).transpose(2, 0, 1, 3)
    ).reshape(P, 8 * N)
    in_maps = [{"x": xh[i], "W": wq} for i in range(N_CORES)]
    nc = _get_nc()
    res = bass_utils.run_bass_kernel_spmd(
        nc, in_maps, core_ids=list(range(N_CORES)), trace=_trace
    )
    out = np.concatenate([r["out"] for r in res.results], axis=0)
    out = out.astype(np.float32).reshape(4, 4096, N)
    if _trace:
        kernel.last_results = res
    return out


# revision 19
# speedup vs baseline: 1.0987x; 1.0855x over previous
"""BinaryLinear Trainium2 kernel: out = sign(x) @ sign(W).T

x: (4, 4096, 1024) f32, W: (1024, 1024) f32 -> out (4, 4096, 1024) f32.

Strategy (8 NeuronCores, data-parallel over flattened batch*seq):
  - Each core gets a [2048, 1024] row-shard of x and the full W.
  - x is re-laid-out on the host (pure permutation, no arithmetic) so the
    contraction index i lands on SBUF partitions directly: per core the DRAM
    tensor is [8 chunks * 128 p, (4 j, 2 c, 2 t', 128 u)] f32 with
    i = 256 j + 128 c + p and row m = 512 g + 4 u + 2 b0 + t' for chunk
    ch = 2 g + b0. This removes the on-chip transpose entirely and loads
    with 8 KiB-per-partition contiguous descriptors.
  - DMA descriptor generation is serialized per queue (~110-140 GB/s per
    queue), so each 1 MiB chunk load is split into 4 sub-DMAs pinned to the
    4 SWDGE queues (qPoolDynamic..qPoolDynamic3). Each queue is FIFO, so
    chunk completions stay ordered without explicit chaining while all 4
    descgen streams run in parallel (~450 GB/s issue capability, HBM-bound).
  - Per chunk (256 rows): ACT Sign (f32 -> fp8e4, +-1/0 exact) -> 16 fp8
    DoubleRow matmuls (K=256 each) accumulating [128 m, 512 o] PSUM tiles
    -> DVE copy PSUM -> SBUF as float16.
  - Outputs are exact integers |v| <= 1024, representable exactly in fp16,
    so stores are half-width; the host upcasts to f32. The 4-way row
    interleave (m = 512g + 4u + b) makes each partition hold 4 adjacent
    DRAM rows, i.e. 8 KiB store descriptors. Stores cover one 512-row
    group (two chunks) and are split into 2 sub-DMAs on the SP and
    Activation HWDGE queues so they never block a load trigger, with the
    Act-queue store triggers reordered after all signs so a store's
    copy-wait can never stall a sign.
  - W is repacked once on the host: wq[p, (j, c, o)] = sign(W)[o, i] fp8;
    loaded as 2 half-DMAs on the SP/Act HWDGE queues concurrently with the
    first x chunk. A dummy 1-element Sign activation with no dependencies
    preloads the ACT function table during the preamble.

All arithmetic is exact: sign values are +-1/0 (exact in fp8e4), the PE
accumulates in fp32, and |out| <= 1024 is exact in fp16.
"""

import numpy as np

P = 128
K = 1024  # in_features
N = 1024  # out_features
N_CORES = 8
M_TOTAL = 4 * 4096
M_PER_CORE = M_TOTAL // N_CORES
MC = 128  # rows per chunk
N_CH = M_PER_CORE // MC
N_GRP = N_CH // 4  # 512-row store groups (4 chunks each)
X_BUFS = 16


def build_binary_linear(tc, out, x, w):
    """Emit the per-core Tile kernel.

    out: DRAM [M_PER_CORE, N] f16, x: DRAM [N_CH*P, 8*MC] f32 (host-packed),
    w: DRAM [P, 8*N] fp8 (host-packed).
    """
    import concourse.mybir as mybir

    nc = tc.nc
    f32 = mybir.dt.float32
    f16 = mybir.dt.float16
    fp8 = mybir.dt.float8e4
    Sign = mybir.ActivationFunctionType.Sign
    DR = mybir.MatmulPerfMode.DoubleRow

    with (
        tc.tile_pool(name="wsb", bufs=1) as wpool,
        tc.tile_pool(name="xin", bufs=X_BUFS) as xin_pool,
        tc.tile_pool(name="x8p", bufs=4) as x8_pool,
        tc.tile_pool(name="osb", bufs=4) as out_pool,
        tc.tile_pool(name="ps", bufs=4, space="PSUM") as psum_pool,
    ):
        # Preload the ACT Sign table during the preamble: a 1-partition,
        # 8-element Sign with no data dependencies.
        dumf = wpool.tile([1, 8], f32, name="dumf")
        dum8 = wpool.tile([1, 8], fp8, name="dum8")
        nc.vector.memset(dumf, 0.0)
        nc.scalar.activation(out=dum8, in_=dumf, func=Sign)

        # ---- W: host-packed fp8 [128, 8*1024]; wq[p, (j, c, o)]
        # = sign(W)[o, i] with i = 256j + 128c + p. Two half-DMAs on the
        # SP / Act HWDGE queues. ----
        wT = wpool.tile([P, 8 * N], fp8, name="wT")
        nc.sync.dma_start(out=wT[: P // 2, :], in_=w[: P // 2, :])
        nc.scalar.dma_start(out=wT[P // 2 :, :], in_=w[P // 2 :, :])
        w4 = wT.rearrange("p (j c o) -> p j c o", j=4, c=2)

        osbs = {}
        for ch in range(N_CH):
            g, b0 = divmod(ch, 4)
            xf = xin_pool.tile([P, 8 * MC], f32, tag="xf", name=f"xf{ch}")
            inst = nc.gpsimd.dma_start(out=xf, in_=x[ch * P : (ch + 1) * P, :])
            qn = 3 if ch % 8 == 7 else ch % 3
            inst.ins.queue = f"qPoolDynamic{qn or ''}"
            x8 = x8_pool.tile([P, 8 * MC], fp8, tag="x8", name=f"x8{ch}")
            nc.scalar.activation(out=x8, in_=xf, func=Sign)
            x84 = x8.rearrange("p (j c m) -> p j c m", j=4, c=2)

            if b0 == 0:
                osbs[g] = out_pool.tile([P, 4 * N], f16, tag="osb", name=f"osb{g}")
            osb2 = osbs[g].rearrange("p (b o) -> p b o", b=4)
            ps = [
                psum_pool.tile([P, 512], f32, tag=f"ps{h}", name=f"ps{h}")
                for h in range(2)
            ]
            for j in range(4):
                lhsT = x84[:, j, :, :]
                for h in range(2):
                    nc.tensor.matmul(
                        ps[h],
                        lhsT=lhsT,
                        rhs=w4[:, j, :, h * 512 : (h + 1) * 512],
                        start=(j == 0),
                        stop=(j == 3),
                        perf_mode=DR,
                    )
            for h in range(2):
                nc.vector.tensor_copy(
                    out=osb2[:, b0, h * 512 : (h + 1) * 512], in_=ps[h]
                )
            # Stores: mid-pipeline groups go on the Pool SWDGE queues as one
            # full-group DMA pair (trigger ~0.6us, descgen offloaded; the
            # ~9.5us completion latency hides behind the PE-paced pipeline).
            # The LAST group's store latency is exposed, so it goes on the
            # engine-synchronous SP/Act HWDGE queues in two b-halves: rows
            # of chunks 12-13 right after their copies (~5us before the
            # end), rows of chunks 14-15 right after the final copies.
            if b0 == 3 and g < N_GRP - 2:
                for q in range(2):
                    r0 = 512 * g + 256 * q
                    o_ap = out[r0 : r0 + 256].rearrange("(p b) o -> p (b o)", b=4)
                    i_ap = osbs[g][64 * q : 64 * (q + 1), :]
                    inst = nc.gpsimd.dma_start(out=o_ap, in_=i_ap)
                    qn = {(0, 0): 3, (0, 1): 3, (1, 0): 0, (1, 1): 1}[(g, q)]
                    inst.ins.queue = f"qPoolDynamic{qn or ''}"
            elif g == N_GRP - 2 and b0 in (1, 3):
                bsl = slice(0, 2) if b0 == 1 else slice(2, 4)
                for q in range(2):
                    r0 = 512 * g + 256 * q
                    o_ap = out[r0 : r0 + 256].rearrange(
                        "(p bb) o -> p bb o", bb=4
                    )[:, bsl, :]
                    i_ap = osbs[g][
                        64 * q : 64 * (q + 1), 2048 * (b0 // 2) : 2048 * (b0 // 2 + 1)
                    ]
                    (nc.sync, nc.scalar)[q].dma_start(out=o_ap, in_=i_ap)
            elif g == N_GRP - 1 and b0 >= 1:
                # final group: per-chunk stores so the piece after the very
                # last copies is only 0.25 MiB
                for q in range(2):
                    r0 = 512 * g + 256 * q
                    o_ap = out[r0 : r0 + 256].rearrange(
                        "(p bb) o -> p bb o", bb=4
                    )[:, b0 : b0 + 1, :]
                    i_ap = osbs[g][
                        64 * q : 64 * (q + 1), 1024 * b0 : 1024 * (b0 + 1)
                    ]
                    (nc.sync, nc.scalar)[q].dma_start(out=o_ap, in_=i_ap)
                if b0 == 1:
                    # chunk 4g+0 rows went unstored above; store them now too
                    for q in range(2):
                        r0 = 512 * g + 256 * q
                        o_ap = out[r0 : r0 + 256].rearrange(
                            "(p bb) o -> p bb o", bb=4
                        )[:, 0:1, :]
                        i_ap = osbs[g][64 * q : 64 * (q + 1), 0:1024]
                        (nc.sync, nc.scalar)[q].dma_start(out=o_ap, in_=i_ap)


def _rewire_waits(nc):
    """Reorder Act-queue store triggers after all signs, then replace Tile's
    conservative / lane-aliased DMA waits with exact producer-based waits.

      xf[ch]     <- sign[ch - X_BUFS] (xf-slot WAR)
      w halves   <- (nothing; first on their HWDGE queues)
      sign[ch]   <- all 4 xf[ch] sub completions (RAW) + keep Tile's PE
                    wait (x8-slot WAR)
      copy[...]  <- keep Tile's PE wait only (psum RAW; osb pool has one
                    buffer per group, no WAR)
      store[g,q] <- last copy of group g (RAW)

    Waits are emitted as (producer's update-sem >= cumulative value after
    it); lane-order waits keep same-sem DMA updates ordered so >= waits
    cannot be satisfied by a later DMA that shares the semaphore.
    """
    import concourse.mybir as mybir

    # -- pass 0a: move Act-engine store DMAs after the last InstActivation --
    for f in nc.m.functions:
        for bb in f.blocks:
            ins_list = bb.instructions
            act_stores = [
                i
                for i in ins_list
                if type(i).__name__ == "InstDMACopy"
                and str(i.engine).endswith("Activation")
                and str(i.outs[0].memref).startswith("out")
            ]
            if not act_stores:
                continue
            rest = [i for i in ins_list if i not in act_stores]
            last_act = max(
                idx
                for idx, i in enumerate(rest)
                if type(i).__name__ == "InstActivation"
            )
            bb.instructions[:] = (
                rest[: last_act + 1] + act_stores + rest[last_act + 1 :]
            )

    # -- pass 0b: reposition Pool-queue store DMAs right after the load
    # trigger whose slot-WAR wait is looser than the store's copy-wait
    # (store[g] after xf[4g+10]), so they never head-of-line block a load --
    for f in nc.m.functions:
        for bb in f.blocks:
            ins_list = bb.instructions
            pool_stores = {}
            for i in ins_list:
                if (
                    type(i).__name__ == "InstDMACopy"
                    and str(i.engine).endswith("Pool")
                    and str(i.outs[0].memref).startswith("out")
                ):
                    g = int(i.outs[0].offset) // (512 * N)
                    pool_stores.setdefault(g, []).append(i)
            if not pool_stores:
                continue
            flat = [i for v in pool_stores.values() for i in v]
            rest = [i for i in ins_list if i not in flat]
            xf_pos = {}
            for idx, i in enumerate(rest):
                if type(i).__name__ == "InstDMACopy" and str(
                    i.outs[0].memref
                ).startswith("xf"):
                    ch = int(str(i.outs[0].memref)[2:].split("_")[0])
                    xf_pos[ch] = idx
            inserts = {}  # position -> [insts]
            last_xf = max(xf_pos.values())
            for g in sorted(pool_stores):
                inserts.setdefault(last_xf, []).extend(pool_stores[g])
            new_list = []
            for idx, i in enumerate(rest):
                new_list.append(i)
                if idx in inserts:
                    new_list.extend(inserts[idx])
            bb.instructions[:] = new_list

    insts = []
    for f in nc.m.functions:
        for bb in f.blocks:
            insts.extend(bb.instructions)

    cum = {}
    upd_after = {}  # inst name -> (sem_name, sem_id, cum_value_after)
    lane_order = {}  # inst name -> SyncWait enforcing same-lane completion order
    xf_subs = {}  # ch -> [inst]
    signs = {}  # ch -> inst
    copies = {}  # g -> [inst]
    stores = {}  # g -> [inst]
    w_loads = []
    for ins in insts:
        si = getattr(ins, "sync_info", None)
        if si is None:
            continue
        for u in si.on_update or []:
            prev = cum.get(u.ant_name, 0)
            if prev > 0 and (
                u.ant_name.startswith("DMAHW") or u.ant_name.startswith("DMASW")
            ):
                lane_order[ins.name] = mybir.SyncWait(
                    sync_type="semaphore",
                    id=u.id,
                    ant_name=u.ant_name,
                    wait_mode="sem-ge-imm",
                    wait_value=prev,
                )
            cum[u.ant_name] = prev + u.update_value
            upd_after[ins.name] = (u.ant_name, u.id, cum[u.ant_name])
        memref = str(getattr(ins.outs[0], "memref", "")) if ins.outs else ""
        tn = type(ins).__name__
        if tn == "InstDMACopy" and memref.startswith("xf"):
            ch = int(memref[2 : memref.index("_")])
            xf_subs.setdefault(ch, []).append(ins)
        elif tn == "InstDMACopy" and memref.startswith("wT"):
            w_loads.append(ins)
        elif tn == "InstDMACopy" and memref.startswith("out"):
            off = int(ins.outs[0].offset)  # in f16 elements
            g = off // (512 * N)
            stores.setdefault(g, []).append(ins)
        elif tn == "InstActivation" and memref.startswith("x8"):
            ch = int(memref[2 : memref.index("_")])
            signs[ch] = ins
        elif tn == "InstTensorCopy" and memref.startswith("osb"):
            g = int(memref[3 : memref.index("_")])
            copies.setdefault(g, []).append(ins)

    assert sorted(xf_subs) == list(range(N_CH)) and all(
        len(v) == 1 for v in xf_subs.values()
    ), {k: len(v) for k, v in xf_subs.items()}
    assert sorted(signs) == list(range(N_CH))
    assert sorted(copies) == list(range(N_GRP)) and all(
        len(v) == 8 for v in copies.values()
    )
    expect = {g: 2 for g in range(N_GRP)}
    expect[N_GRP - 2] = 4
    expect[N_GRP - 1] = 8
    assert {g: len(v) for g, v in stores.items()} == expect, {
        k: len(v) for k, v in stores.items()
    }
    assert len(w_loads) == 2

    def wait_on(producer_ins):
        sem_name, sem_id, v = upd_after[producer_ins.name]
        return mybir.SyncWait(
            sync_type="semaphore",
            id=sem_id,
            ant_name=sem_name,
            wait_mode="sem-ge-imm",
            wait_value=v,
        )

    def keep_engine_waits(ins):
        return [
            w
            for w in (ins.sync_info.on_wait or [])
            if not (
                w.ant_name.startswith("DMAHW")
                or w.ant_name.startswith("DMASW")
                or w.ant_name.startswith("Activation")
                or w.ant_name.startswith("DVE")
            )
        ]

    def set_waits(ins, producers, extra=()):
        si = ins.sync_info
        waits = [wait_on(p) for p in producers if p is not None] + list(extra)
        lo = lane_order.get(ins.name)
        if lo is not None:
            waits.append(lo)
        ins.sync_info = mybir.SyncInfo(
            on_wait=waits, on_update=list(si.on_update or [])
        )

    for ch in range(N_CH):
        for ins in xf_subs[ch]:
            set_waits(ins, [signs[ch - X_BUFS]] if ch >= X_BUFS else [])
    for ins in w_loads:
        set_waits(ins, [])
    for ch in range(N_CH):
        set_waits(signs[ch], xf_subs[ch], extra=keep_engine_waits(signs[ch]))
    for g in range(N_GRP):
        for ins in copies[g]:
            set_waits(ins, [], extra=keep_engine_waits(ins))
        for ins in stores[g]:
            if g == N_GRP - 1:
                b = (int(ins.outs[0].offset) // N) % 4
                dep = copies[g][2 * b + 1]
            elif g == N_GRP - 2:
                half = ((int(ins.outs[0].offset) // N) % 4) // 2
                dep = copies[g][3] if half == 0 else copies[g][7]
            else:
                dep = copies[g][-1]
            set_waits(ins, [dep])


def _legalize_dma_waits(nc):
    """Walrus caps in-struct sem waits (DMA_DIRECT2D takes 1, DMACopy 2).

    Tile's sem assignment is not transitively minimal and can emit 2-4 waits
    on DMA instructions. Hoist the excess into InstEventSemaphore wait-only
    instructions inserted just before the DMA on its triggering queue. This
    is sound: the queue executes the hoisted wait strictly before pushing the
    DMA descriptor, so the dependency is enforced (more conservatively) at
    trigger time instead of ring-pop time.
    """
    import concourse.mybir as mybir

    limits = {
        "InstDmaTransposeAnt": 1,
        "InstDMACopy": 1,
        "InstTensorCopy": 1,
        "InstActivation": 1,
        "InstMatmult": 1,
        "InstLdweights": 1,
        "InstMemset": 1,
        "InstTensorTensor": 1,
        "InstDrain": 1,
    }
    n_hoisted = 0
    for f in nc.m.functions:
        for bb in f.blocks:
            new_list = []
            for ins in bb.instructions:
                lim = limits.get(type(ins).__name__)
                si = getattr(ins, "sync_info", None)
                waits = list(si.on_wait) if si is not None and si.on_wait else []
                if lim is not None and len(waits) > lim:
                    # keep data-producer (engine-sem) waits in-struct first,
                    # then the freshest DMA-lane waits; hoist the rest
                    def keep_rank(w):
                        is_lane = w.ant_name.startswith(
                            "DMAHW"
                        ) or w.ant_name.startswith("DMASW")
                        return (1 if is_lane else 0, -w.wait_value)

                    waits_sorted = sorted(waits, key=keep_rank)
                    keep, hoist = waits_sorted[:lim], waits_sorted[lim:]
                    for ci in range(0, len(hoist), 2):
                        chunk = hoist[ci : ci + 2]
                        ev = mybir.InstEventSemaphore(
                            name=f"{ins.name}-prewait{ci // 2}",
                            engine=ins.engine,
                            ins=[],
                            outs=[],
                            sync_info=mybir.SyncInfo(on_wait=chunk, on_update=[]),
                        )
                        nc.inst_map[ev.name] = ev
                        new_list.append(ev)
                        n_hoisted += len(chunk)
                    ins.sync_info = mybir.SyncInfo(
                        on_wait=keep, on_update=list(si.on_update or [])
                    )
                new_list.append(ins)
            bb.instructions[:] = new_list
    return n_hoisted


def _build_nc():
    import concourse.bass as bass
    import concourse.mybir as mybir
    from concourse import tile

    nc = bass.Bass("TRN2", target_bir_lowering=False, num_swdge_queues=4)
    x_d = nc.dram_tensor(
        "x", [N_CH * P, 8 * MC], mybir.dt.float32, kind="ExternalInput"
    )
    w_d = nc.dram_tensor("W", [P, 8 * N], mybir.dt.float8e4, kind="ExternalInput")
    out_d = nc.dram_tensor(
        "out", [M_PER_CORE, N], mybir.dt.float16, kind="ExternalOutput"
    )
    with tile.TileContext(nc) as tc:
        build_binary_linear(tc, out_d.ap(), x_d.ap(), w_d.ap())
    _rewire_waits(nc)
    _legalize_dma_waits(nc)
    return nc


_cached = {}


def _get_nc():
    if "nc" not in _cached:
        _cached["nc"] = _build_nc()
    return _cached["nc"]


def kernel(x, W, _trace=False):
    from concourse import bass_utils

    import ml_dtypes

    xf = np.asarray(x, dtype=np.float32).reshape(M_TOTAL, K)
    # host re-layout (pure permutation): per core [ (g, b0, p), (j, c, u) ]
    # with m = 2048*core + 512g + 4u + b0 and i = 256j + 128c + p
    T = xf.reshape(N_CORES, 4, P, 4, 4, 2, P)  # (core, g, u, b0, j, c, p)
    xh = np.ascontiguousarray(T.transpose(0, 1, 3, 6, 4, 5, 2)).reshape(
        N_CORES, N_CH * P, 8 * MC
    )
    # pack sign(W) fp8: wq[p, (j, c, o)] = sign(W)[o, 256j + 128c + p]
    sT = np.sign(np.asarray(W, dtype=np.float32)).T.astype(ml_dtypes.float8_e4m3)
    wq = np.ascontiguousarray(
        sT.reshape(4, 2, P, N).transpose(2, 0, 1, 3)
    ).reshape(P, 8 * N)
    in_maps = [{"x": xh[i], "W": wq} for i in range(N_CORES)]
    nc = _get_nc()
    res = bass_utils.run_bass_kernel_spmd(
        nc, in_maps, core_ids=list(range(N_CORES)), trace=_trace
    )
    out = np.concatenate([r["out"] for r in res.results], axis=0)
    out = out.astype(np.float32).reshape(4, 4096, N)
    if _trace:
        kernel.last_results = res
    return out
